# revision 23
# baseline (speedup 1.0000x reference)
"""FastVAR cross-attention block kernel for 8 Trainium2 NeuronCores.

Sharding: 2 batches x 4 head-groups (4 heads each) = 8 cores.
Per-core device program (identical SPMD program, per-core data):
  qkv projection (bf16 matmul, fp32 psum, bias via ones-row K=1 matmul)
  -> l2-normalize q,k (token-major, free-dim reduce)
  -> RoPE (token-major, DVE)
  -> DMA-transpose q,k to feature-major
  -> cos-attention scores (k-major), exp with per-head scale folded in (ACT)
  -> AV with ones-augmented V (softmax denominator for free)
  -> normalize, project (partial over this core's 256 channels)
Host: top-k token selection (replicates reference argsort bitwise on CPU jax),
gather, weight slicing/transposition, partial-sum reduction, scatter + residual.
"""

import math
import os
import sys
from contextlib import ExitStack

import numpy as np

import concourse.bass as bass
import concourse.bacc as bacc
import concourse.tile as tile
from concourse.tile import add_dep_helper
from concourse import mybir
from concourse import bass_utils

# ---------------------------------------------------------------- constants
B = 2
L = 4096
C = 1024
NH = 16
DH = 64
NREM = 1638          # num_remain for this problem
NT = 1664            # padded token count (13 * 128)
HPG = 4              # heads per core (16 heads / 4 groups)
N_CORES = 8

F32 = mybir.dt.float32
BF16 = mybir.dt.bfloat16


class Cfg:
    def __init__(self, NT, NTR, C, HPG, DH, has_bias=True):
        self.NT, self.NTR, self.C, self.HPG, self.DH = NT, NTR, C, HPG, DH
        self.has_bias = has_bias
        self.NC = NT // 128          # token chunks
        self.CH = C // 128           # contraction chunks
        self.QK = 2 * HPG * DH       # q+k feature width
        self.F = 3 * HPG * DH        # qkv feature width
        self.HC = HPG * DH           # head channels per core
        self.HCC = self.HC // 128    # proj contraction chunks


FULL_CFG = Cfg(NT=NT, NTR=NREM, C=C, HPG=HPG, DH=DH)


# ---------------------------------------------------------------- device IR
def emit_core_program(tc, outs, ins, cfg):
    """Emit the per-core Tile program. ins/outs are dicts of DRAM APs."""
    nc = tc.nc
    NTc, NC, Cc, CH, HPGc, DHc = cfg.NT, cfg.NC, cfg.C, cfg.CH, cfg.HPG, cfg.DH
    QK, F, HCC = cfg.QK, cfg.F, cfg.HCC
    NPAIR = max(1, HPGc // 2)
    X = mybir.AxisListType.X

    xmT, wqkvT = ins["xmT"], ins["wqkvT"]
    ropeC, ropeS = ins["ropeC"], ins["ropeS"]
    wpT, scales = ins["wpT"], ins["scales"]
    outp = outs["outp"]

    with ExitStack() as ctx:
        const = ctx.enter_context(tc.tile_pool(name="const", bufs=1))

        # resident inputs
        xm_t = []
        for ci in range(CH):
            t = const.tile([128, NTc], BF16, tag=f"xm{ci}")
            if NTc > 256:
                nc.sync.dma_start(t[:, 0:256], xmT[ci * 128:(ci + 1) * 128, 0:256])
                nc.sync.dma_start(t[:, 256:NTc],
                                  xmT[ci * 128:(ci + 1) * 128, 256:NTc])
            else:
                nc.sync.dma_start(t[:], xmT[ci * 128:(ci + 1) * 128, :])
            xm_t.append(t)
        ones_row = const.tile([1, NTc], BF16, tag="ones_row")
        nc.sync.dma_start(ones_row[:], xmT[Cc:Cc + 1, :])
        w_t = []
        for ci in range(CH):
            t = const.tile([128, F], BF16, tag=f"w{ci}")
            nc.sync.dma_start(t[:], wqkvT[ci * 128:(ci + 1) * 128, :])
            w_t.append(t)
        w_bias = const.tile([1, F], BF16, tag="wb")
        nc.sync.dma_start(w_bias[:], wqkvT[Cc:Cc + 1, :])
        wp_t = []
        for hc in range(HCC):
            t = const.tile([128, Cc], BF16, tag=f"wp{hc}")
            nc.sync.dma_start(t[:], wpT[hc * 128:(hc + 1) * 128, :])
            wp_t.append(t)
        sc_t = const.tile([128, HPGc], F32, tag="scales")
        nc.sync.dma_start(sc_t[:], scales[0:1, :].to_broadcast((128, HPGc)))

        qT_t = [const.tile([128, NTc], BF16, name=f"qT{p}", tag=f"qT{p}") for p in range(NPAIR)]
        kT_t = [const.tile([128, NTc], BF16, name=f"kT{p}", tag=f"kT{p}") for p in range(NPAIR)]
        vav = const.tile([128, NC, HPGc, DHc + 1], BF16, tag="vav")
        oPair = [const.tile([128, NTc], BF16, name=f"oP{i}", tag=f"oP{i}") for i in range(HCC)]

        wk = ctx.enter_context(tc.tile_pool(name="wk", bufs=3))
        pe = ctx.enter_context(tc.tile_pool(name="exp", bufs=6))
        pa = ctx.enter_context(tc.tile_pool(name="att", bufs=2))
        po = ctx.enter_context(tc.tile_pool(name="ob", bufs=3))

        # ---------------- phase 1: qkv + norm + rope + transposes ----------
        with tc.tile_pool(name="p1ps", bufs=3, space="PSUM") as p1:
            for t in range(NC):
                tsl = slice(t * 128, (t + 1) * 128)
                ps = p1.tile([128, F], F32)
                n_ci = CH + 1 if cfg.has_bias else CH
                for ci in range(n_ci):
                    lhs = xm_t[ci][:, tsl] if ci < CH else ones_row[:, tsl]
                    rhsw = w_t[ci] if ci < CH else w_bias
                    for n0 in range(0, F, 512):
                        nn = min(512, F - n0)
                        nc.tensor.matmul(
                            ps[:, n0:n0 + nn], lhs, rhsw[:, n0:n0 + nn],
                            start=(ci == 0), stop=(ci == n_ci - 1),
                        )
                qkv = wk.tile([128, F], F32, tag="qkv")
                nc.scalar.copy(qkv[:], ps[:])

                # l2 norms of q,k along dh: ACT square with accumulate
                nh2 = QK // DHc
                sq = wk.tile([128, DHc], F32, tag="sq")
                ss = wk.tile([128, nh2, 1], F32, tag="ss")
                for hh in range(nh2):
                    nc.scalar.activation(
                        sq[:], qkv[:, hh * DHc:(hh + 1) * DHc],
                        mybir.ActivationFunctionType.Square,
                        accum_out=ss[:, hh, :])
                sroot = wk.tile([128, nh2, 1], F32, tag="sroot")
                nc.scalar.activation(
                    sroot.rearrange("p h one -> p (h one)"),
                    ss.rearrange("p h one -> p (h one)"),
                    mybir.ActivationFunctionType.Sqrt)
                nc.vector.tensor_scalar_max(
                    sroot.rearrange("p h one -> p (h one)"),
                    sroot.rearrange("p h one -> p (h one)"), 1e-12)
                rr = wk.tile([128, nh2, 1], F32, tag="rr")
                nc.vector.reciprocal(
                    rr.rearrange("p h one -> p (h one)"),
                    sroot.rearrange("p h one -> p (h one)"))
                qkn = wk.tile([128, QK], F32, tag="qkn")
                nc.vector.tensor_mul(
                    qkn.rearrange("p (h d) -> p h d", d=DHc),
                    qkv[:, 0:QK].rearrange("p (h d) -> p h d", d=DHc),
                    rr.to_broadcast((128, nh2, DHc)))

                # rope: out_even = c*t_e - s*t_o ; out_odd = s*t_e + c*t_o
                rct = wk.tile([128, 1, DHc], F32, tag="rct")
                nc.gpsimd.dma_start(rct.rearrange("p one d -> p (one d)"), ropeC[tsl, :])
                rst = wk.tile([128, 1, DHc], F32, tag="rst")
                nc.gpsimd.dma_start(rst.rearrange("p one d -> p (one d)"), ropeS[tsl, :])
                ca = wk.tile([128, QK], BF16, tag="ca")
                sa = wk.tile([128, QK], BF16, tag="sa")
                nc.vector.tensor_mul(
                    ca.rearrange("p (h d) -> p h d", d=DHc),
                    qkn.rearrange("p (h d) -> p h d", d=DHc),
                    rct.to_broadcast((128, nh2, DHc)))
                nc.vector.tensor_mul(
                    sa.rearrange("p (h d) -> p h d", d=DHc),
                    qkn.rearrange("p (h d) -> p h d", d=DHc),
                    rst.to_broadcast((128, nh2, DHc)))
                qkr = wk.tile([128, QK], BF16, tag="qkr")

                def ev(tt):
                    return tt.rearrange("p (x two) -> p x two", two=2)[:, :, 0:1]

                def od(tt):
                    return tt.rearrange("p (x two) -> p x two", two=2)[:, :, 1:2]

                nc.vector.tensor_sub(ev(qkr), ev(ca), od(sa))
                nc.vector.tensor_add(od(qkr), ev(sa), od(ca))

                # v (+ softmax-denominator ones column)
                nc.gpsimd.tensor_copy(
                    vav[:, t, :, 0:DHc],
                    qkv[:, QK:F].rearrange("p (h d) -> p h d", d=DHc))
                pad0 = cfg.NTR - (NC - 1) * 128
                if t == NC - 1 and pad0 < 128:
                    nc.vector.memset(vav[:, t, :, DHc:DHc + 1], 0.0)
                    nc.vector.memset(vav[0:pad0, t, :, DHc:DHc + 1], 1.0)
                else:
                    nc.vector.memset(vav[:, t, :, DHc:DHc + 1], 1.0)

                # feature-major q,k via DMA transpose (bf16, 128x128 blocks)
                for j in range(QK // 256):
                    nc.sync.dma_start(
                        qT_t[j][:, tsl], qkr[:, j * 128:(j + 1) * 128],
                        transpose=True)
                for j in range(QK // 256):
                    nc.sync.dma_start(
                        kT_t[j][:, tsl],
                        qkr[:, QK // 2 + j * 128:QK // 2 + (j + 1) * 128],
                        transpose=True)

        # ---------------- phase 2: attention ------------------------------
        # head-pair interleaved: two independent score->exp->AV chains hide
        # the PE<->ACT handoff latency; PSUM = 2 sc bufs + 2 oT accumulators
        QH = NTc // 2
        assert NTc % 128 == 0 and QH % 64 == 0
        QHC = QH // 64
        with tc.tile_pool(name="scps", bufs=2, space="PSUM") as p2, \
             tc.tile_pool(name="avps", bufs=1, space="PSUM") as pav, \
             tc.tile_pool(name="dscr", bufs=2, space="DRAM") as pd:
            for pair in range(NPAIR):
                for qh in range(2):
                    qb = qh * QH
                    oT = [pav.tile([DHc + 1, QH], F32, name=f"oT{i}", tag=f"oT{i}")
                          for i in range(2)]
                    prev = None
                    for kb in range(NC + 1):
                        cur = []
                        last_sc_mm = None
                        if kb < NC:
                            for i in range(2):
                                h = pair * 2 + i
                                kTh = kT_t[pair][i * DHc:(i + 1) * DHc, :]
                                qTh = qT_t[pair][i * DHc:(i + 1) * DHc, :]
                                sc = p2.tile([128, QH], F32, name="sc", tag="sc")
                                for q0 in range(0, QH, 512):
                                    nn = min(512, QH - q0)
                                    last_sc_mm = nc.tensor.matmul(
                                        sc[:, q0:q0 + nn],
                                        kTh[:, kb * 128:(kb + 1) * 128],
                                        qTh[:, qb + q0:qb + q0 + nn],
                                        start=True, stop=True)
                                ex = pe.tile([128, QH], BF16, name="ex", tag="ex")
                                nc.scalar.activation(
                                    ex[:], sc[:],
                                    mybir.ActivationFunctionType.Exp,
                                    scale=sc_t[:, h:h + 1])
                                cur.append(ex)
                        if prev is not None:
                            for i in range(2):
                                h = pair * 2 + i
                                for q0 in range(0, QH, 512):
                                    nn = min(512, QH - q0)
                                    av_mm = nc.tensor.matmul(
                                        oT[i][:, q0:q0 + nn],
                                        vav[:, kb - 1, h, :],
                                        prev[i][:, q0:q0 + nn],
                                        start=(kb - 1 == 0),
                                        stop=(kb - 1 == NC - 1))
                                    if last_sc_mm is not None:
                                        add_dep_helper(
                                            av_mm.ins, last_sc_mm.ins,
                                            sync=False,
                                            reason="keep sc pair adjacent")
                        prev = cur if kb < NC else None
                    # free the PSUM accumulators immediately; normalize
                    # from SBUF copies so the chain overlaps the next section
                    oTs = []
                    for i in range(2):
                        t_sb = pa.tile([DHc + 1, QH], F32, name=f"oTs{i}",
                                       tag=f"oTs{i}")
                        nc.vector.tensor_copy(t_sb[:], oT[i][:])
                        oTs.append(t_sb)
                    for i in range(2):
                        dram_d = pd.tile([1, QH], F32, name="dram_d", tag="dram_d")
                        nc.sync.dma_start(dram_d[:], oTs[i][DHc:DHc + 1, :])
                        den_tok = pa.tile([64, QHC], F32, tag="dtok")
                        nc.sync.dma_start(
                            den_tok[:],
                            dram_d.rearrange("one (c p) -> (one p) c", p=64))
                        rec_tok = pa.tile([64, QHC], F32, tag="rtok")
                        nc.vector.reciprocal(rec_tok[:], den_tok[:])
                        dram_r = pd.tile([QHC, 64], F32, name="dram_r", tag="dram_r")
                        nc.sync.dma_start(dram_r.rearrange("c p -> p c"), rec_tok[:])
                        bc = pa.tile([DHc, QH], F32, tag="bc")
                        bc_src = bass.AP(
                            tensor=dram_r.tensor, offset=dram_r.offset,
                            ap=[[0, DHc], [1, QH]])
                        nc.sync.dma_start(bc[:], bc_src)
                        if i == 0:
                            nc.vector.tensor_mul(
                                oPair[pair][0:DHc, qb:qb + QH],
                                oTs[i][0:DHc, :], bc[:])
                        else:
                            on = pa.tile([DHc, QH], BF16, tag="on")
                            nc.vector.tensor_mul(on[:], oTs[i][0:DHc, :], bc[:])
                            nc.sync.dma_start(
                                oPair[pair][DHc:2 * DHc, qb:qb + QH], on[:])
        # ---------------- phase 3: projection ------------------------------
        with tc.tile_pool(name="p3ps", bufs=2, space="PSUM") as p3:
            for t in range(NC):
                tsl = slice(t * 128, (t + 1) * 128)
                ps = p3.tile([128, Cc], F32)
                for hc in range(HCC):
                    for n0 in range(0, Cc, 512):
                        nn = min(512, Cc - n0)
                        nc.tensor.matmul(
                            ps[:, n0:n0 + nn], oPair[hc][:, tsl],
                            wp_t[hc][:, n0:n0 + nn],
                            start=(hc == 0), stop=(hc == HCC - 1))
                ob = po.tile([128, Cc], F32)
                nc.scalar.copy(ob[:], ps[:])
                nc.sync.dma_start(outp[tsl, :], ob[:])


# ---------------------------------------------------------------- build
def declare_io(nc, cfg):
    ins = {
        "xmT": nc.dram_tensor("xmT", [cfg.C + 1, cfg.NT], BF16,
                              kind="ExternalInput").ap(),
        "wqkvT": nc.dram_tensor("wqkvT", [cfg.C + 1, cfg.F], BF16,
                                kind="ExternalInput").ap(),
        "ropeC": nc.dram_tensor("ropeC", [cfg.NT, cfg.DH], F32,
                                kind="ExternalInput").ap(),
        "ropeS": nc.dram_tensor("ropeS", [cfg.NT, cfg.DH], F32,
                                kind="ExternalInput").ap(),
        "wpT": nc.dram_tensor("wpT", [cfg.HC, cfg.C], BF16,
                              kind="ExternalInput").ap(),
        "scales": nc.dram_tensor("scales", [1, cfg.HPG], F32,
                                 kind="ExternalInput").ap(),
    }
    outs = {
        "outp": nc.dram_tensor("outp", [cfg.NT, cfg.C], F32,
                               kind="ExternalOutput").ap(),
    }
    return ins, outs


_BUILD_CACHE = {}

if os.environ.get("LDW_OPT", "0") == "1":
    _orig_run_command = bass_utils.run_command

    def _patched_run_command(argv, **kw):
        argv = ["--enable-ldw-opt=true" if a == "--enable-ldw-opt=false" else a
                for a in argv]
        return _orig_run_command(argv, **kw)

    bass_utils.run_command = _patched_run_command


def build_full_program(has_bias=True):
    key = ("full", has_bias)
    if key in _BUILD_CACHE:
        return _BUILD_CACHE[key]
    cfg = Cfg(NT=NT, NTR=NREM, C=C, HPG=HPG, DH=DH, has_bias=has_bias)
    nc = bacc.Bacc("TRN2", target_bir_lowering=False, debug=False,
                   num_devices=N_CORES)
    ins, outs = declare_io(nc, cfg)
    with tile.TileContext(nc) as tc:
        emit_core_program(tc, outs, ins, cfg)
    nc.compile()
    _BUILD_CACHE[key] = nc
    return nc


# ---------------------------------------------------------------- host side
def _topk_idx(x, n):
    """Replicate reference token selection exactly (CPU jax; numpy fallback)."""
    try:
        import jax
        import jax.numpy as jnp
        cpu = jax.devices("cpu")[0]
        with jax.default_device(cpu):
            xj = jax.device_put(np.asarray(x), cpu)
            mean = jnp.mean(xj, axis=1, keepdims=True)
            mse = jnp.sum((xj - mean) ** 2, axis=-1)
            idx = jnp.argsort(-mse, axis=1)[:, :n]
            return np.asarray(idx)
    except Exception:
        x = np.asarray(x, np.float32)
        mean = x.mean(1, keepdims=True, dtype=np.float32)
        mse = ((x - mean) ** 2).sum(-1, dtype=np.float32)
        return np.argsort(-mse, axis=1, kind="stable")[:, :n]


def make_in_maps(x, cached_x, W_qkv, q_bias, v_bias, W_proj, b_proj,
                 scale_mul_log, rope_grid, idx, cfg):
    x = np.asarray(x, np.float32)
    W_qkv = np.asarray(W_qkv, np.float32)
    W_proj = np.asarray(W_proj, np.float32)
    q_bias = np.asarray(q_bias, np.float32)
    v_bias = np.asarray(v_bias, np.float32)
    rope_grid = np.asarray(rope_grid, np.float32)
    scale = np.exp(np.minimum(np.asarray(scale_mul_log, np.float32),
                              math.log(100.0))).reshape(NH)

    n_groups = NH // cfg.HPG
    in_maps = []
    per_batch = {}
    for b in range(B):
        xm = x[b, idx[b]]                                   # (NREM, C)
        xmT = np.zeros((cfg.C + 1, cfg.NT), np.float32)
        xmT[:cfg.C, :cfg.NTR] = xm.T
        xmT[cfg.C, :cfg.NTR] = 1.0
        rc = rope_grid[0][idx[b]]                            # (NREM, DH//2)
        rs = rope_grid[1][idx[b]]
        ropeC = np.zeros((cfg.NT, cfg.DH), np.float32)
        ropeS = np.zeros((cfg.NT, cfg.DH), np.float32)
        ropeC[:cfg.NTR] = np.repeat(rc, 2, axis=1)
        ropeS[:cfg.NTR] = np.repeat(rs, 2, axis=1)
        per_batch[b] = (xmT, ropeC, ropeS)

    import ml_dtypes
    bf = ml_dtypes.bfloat16
    for core in range(N_CORES):
        b, hg = divmod(core, n_groups)
        hs = list(range(hg * cfg.HPG, (hg + 1) * cfg.HPG))
        xmT, ropeC, ropeS = per_batch[b]

        wq = np.zeros((cfg.C + 1, cfg.F), np.float32)
        HCb = cfg.HPG * cfg.DH
        for j, h in enumerate(hs):
            rows = slice(h * DH, (h + 1) * DH)
            wq[:cfg.C, j * DH:(j + 1) * DH] = W_qkv[rows, :].T
            wq[:cfg.C, HCb + j * DH:HCb + (j + 1) * DH] = W_qkv[C + h * DH:C + (h + 1) * DH, :].T
            wq[:cfg.C, 2 * HCb + j * DH:2 * HCb + (j + 1) * DH] = W_qkv[2 * C + h * DH:2 * C + (h + 1) * DH, :].T
            wq[cfg.C, j * DH:(j + 1) * DH] = q_bias[h * DH:(h + 1) * DH]
            wq[cfg.C, 2 * HCb + j * DH:2 * HCb + (j + 1) * DH] = v_bias[h * DH:(h + 1) * DH]

        cols = np.concatenate([np.arange(h * DH, (h + 1) * DH) for h in hs])
        wpT = W_proj[:, cols].T.copy()                      # (HC, C)

        in_maps.append({
            "xmT": xmT.astype(bf),
            "wqkvT": wq.astype(bf),
            "ropeC": ropeC,
            "ropeS": ropeS,
            "wpT": wpT.astype(bf),
            "scales": scale[hs].reshape(1, cfg.HPG).astype(np.float32),
        })
    return in_maps


def kernel(x, cached_x, W_qkv, q_bias, v_bias, W_proj, b_proj,
           scale_mul_log, rope_grid, num_remain):
    n = int(num_remain)
    assert n == NREM, f"kernel compiled for num_remain={NREM}, got {n}"
    x = np.asarray(x, np.float32)
    cached_x = np.asarray(cached_x, np.float32)
    b_proj = np.asarray(b_proj, np.float32)

    idx = _topk_idx(x, n)
    cfg = FULL_CFG
    in_maps = make_in_maps(x, cached_x, W_qkv, q_bias, v_bias, W_proj, b_proj,
                           scale_mul_log, rope_grid, idx, cfg)
    has_bias = bool(np.any(np.asarray(q_bias)) or np.any(np.asarray(v_bias)))
    nc = build_full_program(has_bias=has_bias)
    res = bass_utils.run_bass_kernel_spmd(
        nc, in_maps, core_ids=list(range(N_CORES)))
    outs = [np.asarray(r["outp"], np.float32) for r in res.results]

    n_groups = NH // cfg.HPG
    o_full = np.zeros((B, n, C), np.float32)
    for b in range(B):
        acc = outs[b * n_groups][:n]
        for g in range(1, n_groups):
            acc = acc + outs[b * n_groups + g][:n]
        o_full[b] = acc + b_proj

    up = np.broadcast_to(
        cached_x[:, :, None, :, None, :], (B, 32, 2, 32, 2, C)
    ).reshape(B, L, C)
    out = x + up
    bix = np.arange(B)[:, None]
    out[bix, idx] = x[bix, idx] + o_full
    return out.astype(np.float32)


# revision 25
# speedup vs baseline: 1.1178x; 1.1178x over previous
"""FastVAR cross-attention block kernel for 8 Trainium2 NeuronCores.

Sharding: 2 batches x 4 head-groups (4 heads each) = 8 cores.
Per-core device program (identical SPMD program, per-core data):
  qkv projection (bf16 matmul, fp32 psum, bias via ones-row K=1 matmul)
  -> l2-normalize q,k (token-major, free-dim reduce)
  -> RoPE (token-major, DVE)
  -> DMA-transpose q,k to feature-major
  -> cos-attention scores (k-major), exp with per-head scale folded in (ACT)
  -> AV with ones-augmented V (softmax denominator for free)
  -> normalize, project (partial over this core's 256 channels)
Host: top-k token selection (replicates reference argsort bitwise on CPU jax),
gather, weight slicing/transposition, partial-sum reduction, scatter + residual.
"""

import math
import os
import sys
from contextlib import ExitStack

import numpy as np

import concourse.bass as bass
import concourse.bacc as bacc
import concourse.tile as tile
from concourse.tile import add_dep_helper
from concourse import mybir
from concourse import bass_utils

# ---------------------------------------------------------------- constants
B = 2
L = 4096
C = 1024
NH = 16
DH = 64
NREM = 1638          # num_remain for this problem
NT = 1664            # padded token count (13 * 128)
HPG = 4              # heads per core (16 heads / 4 groups)
N_CORES = 8

F32 = mybir.dt.float32
BF16 = mybir.dt.bfloat16


class Cfg:
    def __init__(self, NT, NTR, C, HPG, DH, has_bias=True):
        self.NT, self.NTR, self.C, self.HPG, self.DH = NT, NTR, C, HPG, DH
        self.has_bias = has_bias
        self.NC = NT // 128          # token chunks
        self.CH = C // 128           # contraction chunks
        self.QK = 2 * HPG * DH       # q+k feature width
        self.F = 3 * HPG * DH        # qkv feature width
        self.HC = HPG * DH           # head channels per core
        self.HCC = self.HC // 128    # proj contraction chunks


FULL_CFG = Cfg(NT=NT, NTR=NREM, C=C, HPG=HPG, DH=DH)


# ---------------------------------------------------------------- device IR
def emit_core_program(tc, outs, ins, cfg):
    """Emit the per-core Tile program. ins/outs are dicts of DRAM APs."""
    nc = tc.nc
    NTc, NC, Cc, CH, HPGc, DHc = cfg.NT, cfg.NC, cfg.C, cfg.CH, cfg.HPG, cfg.DH
    QK, F, HCC = cfg.QK, cfg.F, cfg.HCC
    NPAIR = max(1, HPGc // 2)
    X = mybir.AxisListType.X

    xmT, wqkvT = ins["xmT"], ins["wqkvT"]
    ropeC, ropeS = ins["ropeC"], ins["ropeS"]
    wpT, scales = ins["wpT"], ins["scales"]
    outp = outs["outp"]

    with ExitStack() as ctx:
        const = ctx.enter_context(tc.tile_pool(name="const", bufs=1))

        # resident inputs
        xm_t = []
        for ci in range(CH):
            t = const.tile([128, NTc], BF16, tag=f"xm{ci}")
            if NTc > 256:
                nc.gpsimd.dma_start(t[:, 0:256], xmT[ci * 128:(ci + 1) * 128, 0:256])
                nc.gpsimd.dma_start(t[:, 256:NTc],
                                    xmT[ci * 128:(ci + 1) * 128, 256:NTc])
            else:
                nc.gpsimd.dma_start(t[:], xmT[ci * 128:(ci + 1) * 128, :])
            xm_t.append(t)
        ones_row = const.tile([1, NTc], BF16, tag="ones_row")
        nc.sync.dma_start(ones_row[:], xmT[Cc:Cc + 1, :])
        w_t = []
        for ci in range(CH):
            t = const.tile([128, F], BF16, tag=f"w{ci}")
            nc.gpsimd.dma_start(t[:], wqkvT[ci * 128:(ci + 1) * 128, :])
            w_t.append(t)
        w_bias = const.tile([1, F], BF16, tag="wb")
        nc.sync.dma_start(w_bias[:], wqkvT[Cc:Cc + 1, :])
        wp_t = []
        for hc in range(HCC):
            t = const.tile([128, Cc], BF16, tag=f"wp{hc}")
            nc.gpsimd.dma_start(t[:], wpT[hc * 128:(hc + 1) * 128, :])
            wp_t.append(t)
        sc_t = const.tile([128, HPGc], F32, tag="scales")
        nc.sync.dma_start(sc_t[:], scales[0:1, :].to_broadcast((128, HPGc)))
        ident = const.tile([128, 128], BF16, tag="ident")
        nc.gpsimd.dma_start(ident[:], ins["ident"][:])

        qT_t = [const.tile([128, NTc], BF16, name=f"qT{p}", tag=f"qT{p}") for p in range(NPAIR)]
        kT_t = [const.tile([128, NTc], BF16, name=f"kT{p}", tag=f"kT{p}") for p in range(NPAIR)]
        vav = const.tile([128, NC, HPGc, DHc + 1], BF16, tag="vav")
        oPair = [const.tile([128, NTc], BF16, name=f"oP{i}", tag=f"oP{i}") for i in range(HCC)]

        wk = ctx.enter_context(tc.tile_pool(name="wk", bufs=3))
        pe = ctx.enter_context(tc.tile_pool(name="exp", bufs=6))
        pa = ctx.enter_context(tc.tile_pool(name="att", bufs=2))
        po = ctx.enter_context(tc.tile_pool(name="ob", bufs=3))

        # ---------------- phase 1: qkv + norm + rope + transposes ----------
        with tc.tile_pool(name="p1ps", bufs=2, space="PSUM") as p1, \
             tc.tile_pool(name="tpps", bufs=4, space="PSUM") as tp:
            for t in range(NC):
                tsl = slice(t * 128, (t + 1) * 128)
                ps = p1.tile([128, F], F32)
                n_ci = CH + 1 if cfg.has_bias else CH
                for ci in range(n_ci):
                    lhs = xm_t[ci][:, tsl] if ci < CH else ones_row[:, tsl]
                    rhsw = w_t[ci] if ci < CH else w_bias
                    for n0 in range(0, F, 512):
                        nn = min(512, F - n0)
                        nc.tensor.matmul(
                            ps[:, n0:n0 + nn], lhs, rhsw[:, n0:n0 + nn],
                            start=(ci == 0), stop=(ci == n_ci - 1),
                        )
                qkv = wk.tile([128, F], F32, tag="qkv")
                nc.scalar.copy(qkv[:], ps[:])

                # l2 norms of q,k along dh: ACT square with accumulate
                nh2 = QK // DHc
                sq = wk.tile([128, DHc], F32, tag="sq")
                ss = wk.tile([128, nh2, 1], F32, tag="ss")
                for hh in range(nh2):
                    nc.scalar.activation(
                        sq[:], qkv[:, hh * DHc:(hh + 1) * DHc],
                        mybir.ActivationFunctionType.Square,
                        accum_out=ss[:, hh, :])
                sroot = wk.tile([128, nh2, 1], F32, tag="sroot")
                nc.scalar.activation(
                    sroot.rearrange("p h one -> p (h one)"),
                    ss.rearrange("p h one -> p (h one)"),
                    mybir.ActivationFunctionType.Sqrt)
                nc.vector.tensor_scalar_max(
                    sroot.rearrange("p h one -> p (h one)"),
                    sroot.rearrange("p h one -> p (h one)"), 1e-12)
                rr = wk.tile([128, nh2, 1], F32, tag="rr")
                nc.vector.reciprocal(
                    rr.rearrange("p h one -> p (h one)"),
                    sroot.rearrange("p h one -> p (h one)"))
                qkn = wk.tile([128, QK], F32, tag="qkn")
                nc.vector.tensor_mul(
                    qkn.rearrange("p (h d) -> p h d", d=DHc),
                    qkv[:, 0:QK].rearrange("p (h d) -> p h d", d=DHc),
                    rr.to_broadcast((128, nh2, DHc)))

                # rope: out_even = c*t_e - s*t_o ; out_odd = s*t_e + c*t_o
                rct = wk.tile([128, 1, DHc], F32, tag="rct")
                nc.gpsimd.dma_start(rct.rearrange("p one d -> p (one d)"), ropeC[tsl, :])
                rst = wk.tile([128, 1, DHc], F32, tag="rst")
                nc.gpsimd.dma_start(rst.rearrange("p one d -> p (one d)"), ropeS[tsl, :])
                ca = wk.tile([128, QK], BF16, tag="ca")
                sa = wk.tile([128, QK], BF16, tag="sa")
                nc.vector.tensor_mul(
                    ca.rearrange("p (h d) -> p h d", d=DHc),
                    qkn.rearrange("p (h d) -> p h d", d=DHc),
                    rct.to_broadcast((128, nh2, DHc)))
                nc.vector.tensor_mul(
                    sa.rearrange("p (h d) -> p h d", d=DHc),
                    qkn.rearrange("p (h d) -> p h d", d=DHc),
                    rst.to_broadcast((128, nh2, DHc)))
                qkr = wk.tile([128, QK], BF16, tag="qkr")

                def ev(tt):
                    return tt.rearrange("p (x two) -> p x two", two=2)[:, :, 0:1]

                def od(tt):
                    return tt.rearrange("p (x two) -> p x two", two=2)[:, :, 1:2]

                nc.vector.tensor_sub(ev(qkr), ev(ca), od(sa))
                nc.vector.tensor_add(od(qkr), ev(sa), od(ca))

                # v (+ softmax-denominator ones column)
                nc.gpsimd.tensor_copy(
                    vav[:, t, :, 0:DHc],
                    qkv[:, QK:F].rearrange("p (h d) -> p h d", d=DHc))
                pad0 = cfg.NTR - (NC - 1) * 128
                if t == NC - 1 and pad0 < 128:
                    nc.vector.memset(vav[:, t, :, DHc:DHc + 1], 0.0)
                    nc.vector.memset(vav[0:pad0, t, :, DHc:DHc + 1], 1.0)
                else:
                    nc.vector.memset(vav[:, t, :, DHc:DHc + 1], 1.0)

                # feature-major q,k via DMA transpose (bf16, 128x128 blocks)
                for j in range(QK // 128):
                    dst = (qT_t[j][:, tsl] if j < QK // 256 else
                           kT_t[j - QK // 256][:, tsl])
                    tps = tp.tile([128, 128], BF16, name="tps", tag="tps")
                    nc.tensor.transpose(
                        tps[:], qkr[:, j * 128:(j + 1) * 128], ident[:])
                    nc.vector.tensor_copy(dst, tps[:])

        # ---------------- phase 2: attention ------------------------------
        # head-pair interleaved: two independent score->exp->AV chains hide
        # the PE<->ACT handoff latency; PSUM = 2 sc bufs + 2 oT accumulators
        QH = NTc // 2
        assert NTc % 128 == 0 and QH % 64 == 0
        QHC = QH // 64
        with tc.tile_pool(name="scps", bufs=2, space="PSUM") as p2, \
             tc.tile_pool(name="avps", bufs=1, space="PSUM") as pav, \
             tc.tile_pool(name="dscr", bufs=2, space="DRAM") as pd:
            for pair in range(NPAIR):
                for qh in range(2):
                    qb = qh * QH
                    oT = [pav.tile([DHc + 1, QH], F32, name=f"oT{i}", tag=f"oT{i}")
                          for i in range(2)]
                    prev = None
                    for kb in range(NC + 1):
                        cur = []
                        last_sc_mm = None
                        if kb < NC:
                            for i in range(2):
                                h = pair * 2 + i
                                kTh = kT_t[pair][i * DHc:(i + 1) * DHc, :]
                                qTh = qT_t[pair][i * DHc:(i + 1) * DHc, :]
                                sc = p2.tile([128, QH], F32, name="sc", tag="sc")
                                for q0 in range(0, QH, 512):
                                    nn = min(512, QH - q0)
                                    last_sc_mm = nc.tensor.matmul(
                                        sc[:, q0:q0 + nn],
                                        kTh[:, kb * 128:(kb + 1) * 128],
                                        qTh[:, qb + q0:qb + q0 + nn],
                                        start=True, stop=True)
                                ex = pe.tile([128, QH], BF16, name="ex", tag="ex")
                                nc.scalar.activation(
                                    ex[:], sc[:],
                                    mybir.ActivationFunctionType.Exp,
                                    scale=sc_t[:, h:h + 1])
                                cur.append(ex)
                        if prev is not None:
                            for i in range(2):
                                h = pair * 2 + i
                                for q0 in range(0, QH, 512):
                                    nn = min(512, QH - q0)
                                    av_mm = nc.tensor.matmul(
                                        oT[i][:, q0:q0 + nn],
                                        vav[:, kb - 1, h, :],
                                        prev[i][:, q0:q0 + nn],
                                        start=(kb - 1 == 0),
                                        stop=(kb - 1 == NC - 1))
                                    if last_sc_mm is not None:
                                        add_dep_helper(
                                            av_mm.ins, last_sc_mm.ins,
                                            sync=False,
                                            reason="keep sc pair adjacent")
                        prev = cur if kb < NC else None
                    # free the PSUM accumulators immediately; normalize
                    # from SBUF copies so the chain overlaps the next section
                    oTs = []
                    for i in range(2):
                        t_sb = pa.tile([DHc + 1, QH], F32, name=f"oTs{i}",
                                       tag=f"oTs{i}")
                        nc.vector.tensor_copy(t_sb[:], oT[i][:])
                        oTs.append(t_sb)
                    for i in range(2):
                        dram_d = pd.tile([1, QH], F32, name="dram_d", tag="dram_d")
                        nc.sync.dma_start(dram_d[:], oTs[i][DHc:DHc + 1, :])
                        den_tok = pa.tile([64, QHC], F32, tag="dtok")
                        nc.sync.dma_start(
                            den_tok[:],
                            dram_d.rearrange("one (c p) -> (one p) c", p=64))
                        rec_tok = pa.tile([64, QHC], F32, tag="rtok")
                        nc.vector.reciprocal(rec_tok[:], den_tok[:])
                        dram_r = pd.tile([QHC, 64], F32, name="dram_r", tag="dram_r")
                        nc.sync.dma_start(dram_r.rearrange("c p -> p c"), rec_tok[:])
                        bc = pa.tile([DHc, QH], F32, tag="bc")
                        bc_src = bass.AP(
                            tensor=dram_r.tensor, offset=dram_r.offset,
                            ap=[[0, DHc], [1, QH]])
                        nc.sync.dma_start(bc[:], bc_src)
                        if i == 0:
                            nc.vector.tensor_mul(
                                oPair[pair][0:DHc, qb:qb + QH],
                                oTs[i][0:DHc, :], bc[:])
                        else:
                            on = pa.tile([DHc, QH], BF16, tag="on")
                            nc.vector.tensor_mul(on[:], oTs[i][0:DHc, :], bc[:])
                            nc.sync.dma_start(
                                oPair[pair][DHc:2 * DHc, qb:qb + QH], on[:])
        # ---------------- phase 3: projection ------------------------------
        with tc.tile_pool(name="p3ps", bufs=2, space="PSUM") as p3:
            for t in range(NC):
                tsl = slice(t * 128, (t + 1) * 128)
                ps = p3.tile([128, Cc], F32)
                for hc in range(HCC):
                    for n0 in range(0, Cc, 512):
                        nn = min(512, Cc - n0)
                        nc.tensor.matmul(
                            ps[:, n0:n0 + nn], oPair[hc][:, tsl],
                            wp_t[hc][:, n0:n0 + nn],
                            start=(hc == 0), stop=(hc == HCC - 1))
                ob = po.tile([128, Cc], F32)
                nc.scalar.copy(ob[:], ps[:])
                nc.sync.dma_start(outp[tsl, :], ob[:])


# ---------------------------------------------------------------- build
def declare_io(nc, cfg):
    ins = {
        "xmT": nc.dram_tensor("xmT", [cfg.C + 1, cfg.NT], BF16,
                              kind="ExternalInput").ap(),
        "wqkvT": nc.dram_tensor("wqkvT", [cfg.C + 1, cfg.F], BF16,
                                kind="ExternalInput").ap(),
        "ropeC": nc.dram_tensor("ropeC", [cfg.NT, cfg.DH], F32,
                                kind="ExternalInput").ap(),
        "ropeS": nc.dram_tensor("ropeS", [cfg.NT, cfg.DH], F32,
                                kind="ExternalInput").ap(),
        "wpT": nc.dram_tensor("wpT", [cfg.HC, cfg.C], BF16,
                              kind="ExternalInput").ap(),
        "scales": nc.dram_tensor("scales", [1, cfg.HPG], F32,
                                 kind="ExternalInput").ap(),
        "ident": nc.dram_tensor("ident", [128, 128], BF16,
                                kind="ExternalInput").ap(),
    }
    outs = {
        "outp": nc.dram_tensor("outp", [cfg.NT, cfg.C], F32,
                               kind="ExternalOutput").ap(),
    }
    return ins, outs


_BUILD_CACHE = {}

if os.environ.get("LDW_OPT", "0") == "1":
    _orig_run_command = bass_utils.run_command

    def _patched_run_command(argv, **kw):
        argv = ["--enable-ldw-opt=true" if a == "--enable-ldw-opt=false" else a
                for a in argv]
        return _orig_run_command(argv, **kw)

    bass_utils.run_command = _patched_run_command


def build_full_program(has_bias=True):
    key = ("full", has_bias)
    if key in _BUILD_CACHE:
        return _BUILD_CACHE[key]
    cfg = Cfg(NT=NT, NTR=NREM, C=C, HPG=HPG, DH=DH, has_bias=has_bias)
    nc = bacc.Bacc("TRN2", target_bir_lowering=False, debug=False,
                   num_devices=N_CORES)
    ins, outs = declare_io(nc, cfg)
    with tile.TileContext(nc) as tc:
        emit_core_program(tc, outs, ins, cfg)
    nc.compile()
    _BUILD_CACHE[key] = nc
    return nc


# ---------------------------------------------------------------- host side
def _topk_idx(x, n):
    """Replicate reference token selection exactly (CPU jax; numpy fallback)."""
    try:
        import jax
        import jax.numpy as jnp
        cpu = jax.devices("cpu")[0]
        with jax.default_device(cpu):
            xj = jax.device_put(np.asarray(x), cpu)
            mean = jnp.mean(xj, axis=1, keepdims=True)
            mse = jnp.sum((xj - mean) ** 2, axis=-1)
            idx = jnp.argsort(-mse, axis=1)[:, :n]
            return np.asarray(idx)
    except Exception:
        x = np.asarray(x, np.float32)
        mean = x.mean(1, keepdims=True, dtype=np.float32)
        mse = ((x - mean) ** 2).sum(-1, dtype=np.float32)
        return np.argsort(-mse, axis=1, kind="stable")[:, :n]


def make_in_maps(x, cached_x, W_qkv, q_bias, v_bias, W_proj, b_proj,
                 scale_mul_log, rope_grid, idx, cfg):
    x = np.asarray(x, np.float32)
    W_qkv = np.asarray(W_qkv, np.float32)
    W_proj = np.asarray(W_proj, np.float32)
    q_bias = np.asarray(q_bias, np.float32)
    v_bias = np.asarray(v_bias, np.float32)
    rope_grid = np.asarray(rope_grid, np.float32)
    scale = np.exp(np.minimum(np.asarray(scale_mul_log, np.float32),
                              math.log(100.0))).reshape(NH)

    n_groups = NH // cfg.HPG
    in_maps = []
    per_batch = {}
    for b in range(B):
        xm = x[b, idx[b]]                                   # (NREM, C)
        xmT = np.zeros((cfg.C + 1, cfg.NT), np.float32)
        xmT[:cfg.C, :cfg.NTR] = xm.T
        xmT[cfg.C, :cfg.NTR] = 1.0
        rc = rope_grid[0][idx[b]]                            # (NREM, DH//2)
        rs = rope_grid[1][idx[b]]
        ropeC = np.zeros((cfg.NT, cfg.DH), np.float32)
        ropeS = np.zeros((cfg.NT, cfg.DH), np.float32)
        ropeC[:cfg.NTR] = np.repeat(rc, 2, axis=1)
        ropeS[:cfg.NTR] = np.repeat(rs, 2, axis=1)
        per_batch[b] = (xmT, ropeC, ropeS)

    import ml_dtypes
    bf = ml_dtypes.bfloat16
    for core in range(N_CORES):
        b, hg = divmod(core, n_groups)
        hs = list(range(hg * cfg.HPG, (hg + 1) * cfg.HPG))
        xmT, ropeC, ropeS = per_batch[b]

        wq = np.zeros((cfg.C + 1, cfg.F), np.float32)
        HCb = cfg.HPG * cfg.DH
        for j, h in enumerate(hs):
            rows = slice(h * DH, (h + 1) * DH)
            wq[:cfg.C, j * DH:(j + 1) * DH] = W_qkv[rows, :].T
            wq[:cfg.C, HCb + j * DH:HCb + (j + 1) * DH] = W_qkv[C + h * DH:C + (h + 1) * DH, :].T
            wq[:cfg.C, 2 * HCb + j * DH:2 * HCb + (j + 1) * DH] = W_qkv[2 * C + h * DH:2 * C + (h + 1) * DH, :].T
            wq[cfg.C, j * DH:(j + 1) * DH] = q_bias[h * DH:(h + 1) * DH]
            wq[cfg.C, 2 * HCb + j * DH:2 * HCb + (j + 1) * DH] = v_bias[h * DH:(h + 1) * DH]

        cols = np.concatenate([np.arange(h * DH, (h + 1) * DH) for h in hs])
        wpT = W_proj[:, cols].T.copy()                      # (HC, C)

        in_maps.append({
            "ident": np.eye(128, dtype=np.float32).astype(bf),
            "xmT": xmT.astype(bf),
            "wqkvT": wq.astype(bf),
            "ropeC": ropeC,
            "ropeS": ropeS,
            "wpT": wpT.astype(bf),
            "scales": scale[hs].reshape(1, cfg.HPG).astype(np.float32),
        })
    return in_maps


def kernel(x, cached_x, W_qkv, q_bias, v_bias, W_proj, b_proj,
           scale_mul_log, rope_grid, num_remain):
    n = int(num_remain)
    assert n == NREM, f"kernel compiled for num_remain={NREM}, got {n}"
    x = np.asarray(x, np.float32)
    cached_x = np.asarray(cached_x, np.float32)
    b_proj = np.asarray(b_proj, np.float32)

    idx = _topk_idx(x, n)
    cfg = FULL_CFG
    in_maps = make_in_maps(x, cached_x, W_qkv, q_bias, v_bias, W_proj, b_proj,
                           scale_mul_log, rope_grid, idx, cfg)
    has_bias = bool(np.any(np.asarray(q_bias)) or np.any(np.asarray(v_bias)))
    nc = build_full_program(has_bias=has_bias)
    res = bass_utils.run_bass_kernel_spmd(
        nc, in_maps, core_ids=list(range(N_CORES)))
    outs = [np.asarray(r["outp"], np.float32) for r in res.results]

    n_groups = NH // cfg.HPG
    o_full = np.zeros((B, n, C), np.float32)
    for b in range(B):
        acc = outs[b * n_groups][:n]
        for g in range(1, n_groups):
            acc = acc + outs[b * n_groups + g][:n]
        o_full[b] = acc + b_proj

    up = np.broadcast_to(
        cached_x[:, :, None, :, None, :], (B, 32, 2, 32, 2, C)
    ).reshape(B, L, C)
    out = x + up
    bix = np.arange(B)[:, None]
    out[bix, idx] = x[bix, idx] + o_full
    return out.astype(np.float32)
